# revision 14
# baseline (speedup 1.0000x reference)
"""CQAttention (QANet context-query attention) Trainium2 kernel.

Problem: B=64, H=256, Lc=2048, Lq=256.
  S[b,i,j] = (Ct@w1)[i] + (Qt@w2)[j] + sum_h Ct[i,h]*w3[h]*Qt[j,h]
  S_row = softmax_j(masked), S_col = softmax_i(masked)
  A = S_row @ Qt ; Bt = S_row @ (S_col^T @ Ct)
  out[b] = [Ct; A; Ct*A; Ct*Bt]^T  -> [B, 4H, Lc]

Strategy: data-parallel over batch (8 per core x 8 cores). Per batch:
  - host precomputes mask-folded bias column cb, fp16 C, fp16 Q*w3,
    fp16 Q^T, and bf16 g-scaled Ct_aug = [g*Ct | g] with
    g[i] = exp(r[i] - 1e30*cmask[i]); all SBUF-packed.
  - S^T [j,i] on PE (lhsT=Q*w3, rhs=C) -> ACT exp with per-partition bias
    cb[j] -> Pr^T bf16 (unnormalized).
  - row sums replicated across partitions via ones-matmul; reciprocal on
    DVE -> rrep bf16; prn = Pr^T * rrep -> bf16 (row-normalized).
  - col-path matrix from PE transposes of Pr^T: since
    exp(S+rm[i]) = exp(S^T+cb[j])^T * exp(rm[i]) * exp(-cb[j]) and the
    exp(-cb[j]) factor cancels in the column softmax, X comes from the
    g-folded Ct_aug: X_aug = Pc0^T @ [g*Ct | g] -> numerator + colsum;
    normalized on eviction (tensor_scalar by 1/(colsum+eps)) -> fp16.
  - A^T = Qt^T @ prn and Bt^T = X^T @ prn, already normalized.
  - epilogue: o2=A^T copy (ACT), o3=A^T*C and o4=Bt^T*C from PSUM (DVE);
    fp16 [128,2048] staging, 6 x 0.5MB output DMAs per batch.
  - host assembles: section 0 = C (identity), sections 1-3 cast fp16->f32.
"""

import numpy as np

B, H, LC, LQ = 64, 256, 2048, 256
NCORES = 8
NB = B // NCORES  # batches per core
NEG = 1.0e30

HC = H // 128   # 2 h-chunks
JC = LQ // 128  # 2 j-chunks
IC = LC // 128  # 16 i-chunks
IT = LC // 512  # 4 i-tiles
HA = H + 1      # augmented (g column) width

_CACHE = {}


def _build():
    import concourse.bacc as bacc
    import concourse.mybir as mybir
    import concourse.tile as tile
    from contextlib import ExitStack

    F32 = mybir.dt.float32
    F16 = mybir.dt.float16
    BF16 = mybir.dt.bfloat16
    AF = mybir.ActivationFunctionType
    MUL = mybir.AluOpType.mult

    nc = bacc.Bacc("TRN2", target_bir_lowering=False, debug=False,
                   enable_asserts=False)

    c32 = nc.dram_tensor("c32", [NB, 128, HC * LC], F16, kind="ExternalInput").ap()
    q3 = nc.dram_tensor("q3", [NB, 128, HC * LQ], F16, kind="ExternalInput").ap()
    qt = nc.dram_tensor("qt", [NB, 128, JC * H], F16, kind="ExternalInput").ap()
    cta = nc.dram_tensor("cta", [NB, 128, IC * HA], BF16, kind="ExternalInput").ap()
    rcb = nc.dram_tensor("rcb", [NB, 128, JC], F32, kind="ExternalInput").ap()
    out = nc.dram_tensor("out", [NB, 3, 2 * 128, LC], F16, kind="ExternalOutput").ap()

    with tile.TileContext(nc) as tc:
        with ExitStack() as ctx:
            konst = ctx.enter_context(tc.tile_pool(name="konst", bufs=1))
            crpool = ctx.enter_context(tc.tile_pool(name="crpool", bufs=2))
            ctpool = ctx.enter_context(tc.tile_pool(name="ctpool", bufs=2))
            qpool = ctx.enter_context(tc.tile_pool(name="qpool", bufs=3))
            prpool = ctx.enter_context(tc.tile_pool(name="prpool", bufs=2))
            prnpool = ctx.enter_context(tc.tile_pool(name="prnpool", bufs=2))
            pcpool = ctx.enter_context(tc.tile_pool(name="pcpool", bufs=2))
            rrpool = ctx.enter_context(tc.tile_pool(name="rrpool", bufs=2))
            xpool = ctx.enter_context(tc.tile_pool(name="xpool", bufs=2))
            opool = ctx.enter_context(tc.tile_pool(name="opool", bufs=6))
            small = ctx.enter_context(tc.tile_pool(name="small", bufs=8))
            mm_ps = ctx.enter_context(tc.tile_pool(name="mm_ps", bufs=5, space="PSUM"))
            x_ps = ctx.enter_context(tc.tile_pool(name="x_ps", bufs=2, space="PSUM"))

            ones32 = konst.tile([128, 128], F32)
            nc.vector.memset(ones32[:], 1.0)
            ones_b = konst.tile([128, 128], BF16)
            nc.vector.tensor_copy(ones_b[:], ones32[:])
            eps = konst.tile([128, 1], F32)
            nc.vector.memset(eps[:], 1e-30)

            def load_batch(b):
                q3sb = qpool.tile([128, HC * LQ], F16, tag="q3sb")
                nc.sync.dma_start(q3sb[:], q3[b])
                crsb = crpool.tile([128, HC * LC], F16, tag="crsb")
                nc.sync.dma_start(crsb[:], c32[b])
                qtsb = qpool.tile([128, JC * H], F16, tag="qtsb")
                nc.sync.dma_start(qtsb[:], qt[b])
                ctsb = ctpool.tile([128, IC * HA], BF16, tag="ctsb")
                nc.sync.dma_start(ctsb[:], cta[b])
                rcbsb = small.tile([128, JC], F32, tag="rcbsb")
                nc.sync.dma_start(rcbsb[:], rcb[b])
                return crsb, q3sb, qtsb, ctsb, rcbsb

            tiles = load_batch(0)
            for b in range(NB):
                crsb, q3sb, qtsb, ctsb, cbsb = tiles
                cf = crsb[:]  # fp16 C for the epilogue products
                if b + 1 < NB:
                    tiles = load_batch(b + 1)

                # ---- row path: S^T tiles -> exp -> Pr^T bf16; rowsums ----
                prt = prpool.tile([128, JC * LC], BF16, tag="prt")
                prn = prnpool.tile([128, JC * LC], BF16, tag="prn")
                rrep = rrpool.tile([128, LC], BF16, tag="rrep")
                for it in range(IT):
                    for jc in range(JC):
                        ps = mm_ps.tile([128, 512], F32, tag="mm")
                        for kc in range(HC):
                            nc.tensor.matmul(
                                ps[:],
                                q3sb[:, kc * LQ + jc * 128:kc * LQ + (jc + 1) * 128],
                                crsb[:, kc * LC + it * 512:kc * LC + (it + 1) * 512],
                                start=(kc == 0), stop=(kc == HC - 1))
                        nc.scalar.activation(
                            prt[:, jc * LC + it * 512:jc * LC + (it + 1) * 512],
                            ps[:], AF.Exp, bias=cbsb[:, jc:jc + 1])
                    rs = mm_ps.tile([128, 512], F32, tag="mm")
                    for jc in range(JC):
                        nc.tensor.matmul(
                            rs[:], ones_b[:],
                            prt[:, jc * LC + it * 512:jc * LC + (it + 1) * 512],
                            start=(jc == 0), stop=(jc == JC - 1))
                    rr32 = small.tile([128, 512], F32, tag="rr32")
                    nc.vector.reciprocal_approx_fast(rr32[:], rs[:])
                    nc.scalar.copy(rrep[:, it * 512:(it + 1) * 512], rr32[:])
                # normalized bf16 Pr^T (per jc chunk, 2048-wide DVE 4x ops)
                for jc in range(JC):
                    nc.vector.tensor_tensor(
                        prn[:, jc * LC:(jc + 1) * LC],
                        prt[:, jc * LC:(jc + 1) * LC], rrep[:], MUL)

                # ---- col path: XBAR DMA transposes of Pr^T -> Pc0 [i, j] ----
                pc = pcpool.tile([128, IC * LQ], BF16, tag="pc")
                pc3 = pc[:].rearrange("p (c j) -> p c j", c=IC)
                for jc in range(JC):
                    nc.sync.dma_start(
                        pc3[:, :, jc * 128:(jc + 1) * 128],
                        prt[:, jc * LC:(jc + 1) * LC], transpose=True)

                # ---- M3: X_aug = Pc0^T @ [g*Ct|g]; normalize by colsum ----
                xsb = xpool.tile([128, JC * H], F16, tag="xsb")
                for jc in range(JC):
                    xps = x_ps.tile([128, HA], F32, tag="x")
                    for ic in range(IC):
                        nc.tensor.matmul(
                            xps[:],
                            pc[:, ic * LQ + jc * 128:ic * LQ + (jc + 1) * 128],
                            ctsb[:, ic * HA:(ic + 1) * HA],
                            start=(ic == 0), stop=(ic == IC - 1))
                    cse = small.tile([128, 1], F32, tag="cse")
                    nc.vector.tensor_tensor(cse[:], xps[:, H:H + 1], eps[:],
                                            mybir.AluOpType.add)
                    colr = small.tile([128, 1], F32, tag="colr")
                    nc.vector.reciprocal_approx_fast(colr[:], cse[:])
                    nc.vector.tensor_scalar_mul(
                        xsb[:, jc * H:(jc + 1) * H], xps[:, 0:H], colr[:])

                # ---- M2/M4 + epilogue ----
                for hc in range(HC):
                    o2 = opool.tile([128, LC], F16, tag="obuf")
                    o3 = opool.tile([128, LC], F16, tag="obuf")
                    o4 = opool.tile([128, LC], F16, tag="obuf")
                    for it in range(IT):
                        i0, i1 = it * 512, (it + 1) * 512
                        aps = mm_ps.tile([128, 512], F32, tag="mm")
                        for jc in range(JC):
                            nc.tensor.matmul(
                                aps[:],
                                qtsb[:, jc * H + hc * 128:jc * H + (hc + 1) * 128],
                                prn[:, jc * LC + i0:jc * LC + i1],
                                start=(jc == 0), stop=(jc == JC - 1))
                        bps = mm_ps.tile([128, 512], F32, tag="mm")
                        for jc in range(JC):
                            nc.tensor.matmul(
                                bps[:],
                                xsb[:, jc * H + hc * 128:jc * H + (hc + 1) * 128],
                                prn[:, jc * LC + i0:jc * LC + i1],
                                start=(jc == 0), stop=(jc == JC - 1))
                        # O2 = A^T ; O3 = A^T*C ; O4 = Bt^T*C
                        nc.scalar.copy(o2[:, i0:i1], aps[:])
                        nc.vector.tensor_tensor(
                            o3[:, i0:i1], aps[:],
                            cf[:, hc * LC + i0:hc * LC + i1], MUL)
                        nc.vector.tensor_tensor(
                            o4[:, i0:i1], bps[:],
                            cf[:, hc * LC + i0:hc * LC + i1], MUL)
                    nc.sync.dma_start(out[b, 0, hc * 128:(hc + 1) * 128, :], o2[:])
                    nc.sync.dma_start(out[b, 1, hc * 128:(hc + 1) * 128, :], o3[:])
                    nc.sync.dma_start(out[b, 2, hc * 128:(hc + 1) * 128, :], o4[:])

    nc.compile()
    return nc


def _prep(C, Q, cmask, qmask, line_project):
    import ml_dtypes
    w1, w2, w3 = np.split(line_project.astype(np.float64), 3)
    r = np.einsum('bhi,h->bi', C.astype(np.float64), w1).astype(np.float32)
    c_ = np.einsum('bhj,h->bj', Q.astype(np.float64), w2).astype(np.float32)
    g = np.exp(r) * (1.0 - cmask)  # exp(r - 1e30*cmask)
    cb = (c_ - NEG * qmask).reshape(B, JC, 128).transpose(0, 2, 1)
    cb = np.ascontiguousarray(cb).astype(np.float32)
    q3 = (Q * w3.astype(np.float32)[None, :, None]).astype(np.float16)
    q3 = q3.reshape(B, HC, 128, LQ).transpose(0, 2, 1, 3).reshape(B, 128, HC * LQ)
    qt = Q.transpose(0, 2, 1).astype(np.float16)
    qt = qt.reshape(B, JC, 128, H).transpose(0, 2, 1, 3).reshape(B, 128, JC * H)
    cta = np.empty((B, LC, HA), dtype=np.float32)
    cta[:, :, 0:H] = C.transpose(0, 2, 1) * g[:, :, None]
    cta[:, :, H] = g
    cta = cta.astype(ml_dtypes.bfloat16)
    cta = cta.reshape(B, IC, 128, HA).transpose(0, 2, 1, 3).reshape(B, 128, IC * HA)
    c16 = C.astype(np.float16).reshape(B, HC, 128, LC) \
        .transpose(0, 2, 1, 3).reshape(B, 128, HC * LC)
    return cb, q3, qt, cta, c16


def make_in_maps(C, Q, cmask, qmask, line_project):
    import ml_dtypes
    C = np.asarray(C, dtype=np.float32)
    Q = np.asarray(Q, dtype=np.float32)
    cmask = np.asarray(cmask, dtype=np.float32)
    qmask = np.asarray(qmask, dtype=np.float32)
    line_project = np.asarray(line_project, dtype=np.float32)
    cb, q3, qt, cta, c16 = _prep(C, Q, cmask, qmask, line_project)
    in_maps = []
    for core in range(NCORES):
        s = slice(core * NB, (core + 1) * NB)
        in_maps.append({
            "c32": np.ascontiguousarray(c16[s]),
            "q3": np.ascontiguousarray(q3[s]),
            "qt": np.ascontiguousarray(qt[s]),
            "cta": np.ascontiguousarray(cta[s]),
            "rcb": np.ascontiguousarray(cb[s]),
        })
    return in_maps


def kernel(C, Q, cmask, qmask, line_project):
    from concourse.bass_utils import run_bass_kernel_spmd

    C = np.asarray(C, dtype=np.float32)
    in_maps = make_in_maps(C, Q, cmask, qmask, line_project)
    if "nc" not in _CACHE:
        _CACHE["nc"] = _build()
    nc = _CACHE["nc"]
    res = run_bass_kernel_spmd(nc, in_maps, core_ids=list(range(NCORES)))
    _CACHE["last_results"] = res
    dev = np.concatenate([res.results[c]["out"] for c in range(NCORES)], axis=0)
    full = np.empty((B, 4 * H, LC), dtype=np.float32)
    full[:, 0:H, :] = C
    full[:, H:2 * H, :] = dev[:, 0].astype(np.float32)
    full[:, 2 * H:3 * H, :] = dev[:, 1].astype(np.float32)
    full[:, 3 * H:4 * H, :] = dev[:, 2].astype(np.float32)
    return full
